# revision 44
# baseline (speedup 1.0000x reference)
"""Trainium2 Bass kernel for nn_Attention_49005576847767.

GQA attention block (QKV proj + Q/K RMSNorm + NeoX RoPE + sliding-window
causal attention with tanh softcap + output proj), tensor-parallel over
heads across 8 NeuronCores.

Sharding: core c owns KV head c and query heads 4c..4c+3.
  Merged stage 1+2: per 128-row s-tile, QKV projection (fp32r matmuls),
    RMSNorm + RoPE epilogue, PE transposes -> qT/kT/v; after every odd
    s-tile, flash-style attention for the finished 256-row q-chunk with
    *transposed* scores [s_k, s_q] (softcap bounds scores at +-50 so no
    max-subtraction is needed; row sums via a ones-column matmul).
    Interleaving keeps TensorE busy while ScalarE does tanh/exp.
  Stage 3: AllToAll reshards o from head-split to sequence-split, then
    each core computes its 256 output rows against the full wo (bf16).
Host assembles the 8 row-shards.
"""

import numpy as np

import concourse.bass as bass
import concourse.mybir as mybir
import concourse.tile as tile
from concourse import bacc
from concourse.bass_utils import run_bass_kernel_spmd
from concourse.masks import make_identity

F32 = mybir.dt.float32
F32R = mybir.dt.float32r
BF16 = mybir.dt.bfloat16
AF = mybir.ActivationFunctionType
ALU = mybir.AluOpType

# problem shapes (hardcoded per contract)
B, S, H = 1, 2048, 4096
HQ, HKV, D = 32, 8, 128
NC = 8                 # cores
NH = HQ // NC          # 4 query heads per core
WINDOW = 1024
SOFTCAP = 50.0
EPS = 1e-6
THETA = 10000.0
SCALE = 1.0 / float(np.sqrt(np.float32(D)))

ST = S // 128          # 16 s-tiles
NK = H // 128          # 32 contraction tiles for projections
CH = S // 256          # 8 q-chunks of 256 rows
SSH = S // NC          # 256 output rows per core

MASK_SLOT = {-8: 0, -7: 1, 0: 2, 1: 3}


def _round_f32r(x: np.ndarray) -> np.ndarray:
    """Round fp32 to the PE's fp32r format (RNE, 12 mantissa bits dropped)."""
    u = np.ascontiguousarray(x.astype(np.float32)).view(np.uint32)
    u = (u + 0x7FF + ((u >> 12) & 1)) & np.uint32(0xFFFFF000)
    return u.view(np.float32)


def _rope_tables():
    half = D // 2
    inv_freq = 1.0 / (THETA ** (np.arange(half, dtype=np.float64) / half))
    ang = np.arange(S, dtype=np.float64)[:, None] * inv_freq[None, :]
    return (np.cos(ang).astype(np.float32), np.sin(ang).astype(np.float32))


def _mask_tiles() -> np.ndarray:
    """[4, 128, 256] multiplicative masks for relative k-tile offsets
    r in {-8, -7, 0, +1} of a 256-wide q-chunk. Entry [b, a] valid iff
    0 <= a - b - 128 r <= WINDOW."""
    b = np.arange(128)[:, None]
    a = np.arange(256)[None, :]
    out = np.zeros((4, 128, 256), np.float32)
    for idx, r in enumerate((-8, -7, 0, 1)):
        d = a - b - 128 * r
        out[idx] = ((d >= 0) & (d <= WINDOW)).astype(np.float32)
    return out


def build_program(reps: int = 0, sim_mode: bool = False, stages=(1, 2, 3),
                  timing_mode: bool = False, ablate=frozenset(), knobs=None):
    """Build the SPMD program. reps=0 -> straight-line (graded path);
    reps=N>0 -> static hardware loops; reps=-1 -> loop count read from a
    uint32 input at runtime (timing). sim_mode -> single-core, collective
    replaced by a local DMA, for cost-model runs."""
    stages = set(stages)
    kn = {"xa_bufs": 2, "sc_bufs": 2, "s2sb_bufs": 3, "wo_bufs": 12,
          "wqkv_chunks": 8, "gp_bcast": True, "dve_epi": False, "t_bufs": 1, "o_bufs": 2}
    kn.update(knobs or {})
    nc = bacc.Bacc("TRN2", target_bir_lowering=False, debug=False,
                   num_devices=1 if sim_mode else NC)

    if timing_mode:
        # garbage-valued internal tensors: no host->device transfer, so
        # per-call wall is RTT + R * kernel-time (values don't affect timing)
        xT = nc.dram_tensor("xT", [H, S], F32R).ap()
        wqkv = nc.dram_tensor("wqkv", [H, 768], F32R).ap()
        wo = nc.dram_tensor("wo", [H, H], BF16).ap()
    else:
        xT = nc.dram_tensor("xT", [H, S], F32R, kind="ExternalInput").ap()
        wqkv = nc.dram_tensor("wqkv", [H, 768], F32R, kind="ExternalInput").ap()
        wo = nc.dram_tensor("wo", [H, H], BF16, kind="ExternalInput").ap()
    cos_in = nc.dram_tensor("cos_in", [S, 64], F32, kind="ExternalInput").ap()
    sin_in = nc.dram_tensor("sin_in", [S, 64], F32, kind="ExternalInput").ap()
    masks_in = nc.dram_tensor("masks_in", [4, 128, 256], F32R,
                              kind="ExternalInput").ap()
    qw_in = nc.dram_tensor("qw_in", [1, D], F32, kind="ExternalInput").ap()
    kw_in = nc.dram_tensor("kw_in", [1, D], F32, kind="ExternalInput").ap()
    ones_in = nc.dram_tensor("ones_in", [128, 128], F32R,
                             kind="ExternalInput").ap()
    if reps == -1:
        reps_in = nc.dram_tensor("reps_in", [1, 1], mybir.dt.uint32,
                                 kind="ExternalInput").ap()
    if timing_mode:
        out_shard = nc.dram_tensor("out_shard", [SSH, H], F32).ap()
        tiny_out = nc.dram_tensor("tiny_out", [16, 64], F32,
                                  kind="ExternalOutput").ap()
    else:
        out_shard = nc.dram_tensor("out_shard", [SSH, H], F32,
                                   kind="ExternalOutput").ap()
        tiny_out = None

    a2a_in = nc.dram_tensor("a2a_in", [NC, NH * D, SSH], BF16)
    a2a_out = nc.dram_tensor("a2a_out", [NC, NH * D, SSH], BF16)

    with tile.TileContext(nc) as tc:
        with tc.tile_pool(name="const", bufs=1) as cpool:
            # ---- constants (~15KB/partition) ----
            ident = cpool.tile([128, 128], F32)
            make_identity(nc, ident[:])
            ones = cpool.tile([128, 128], F32R)
            nc.sync.dma_start(out=ones[:], in_=ones_in)
            masks = cpool.tile([128, 4 * 256], F32R)
            nc.sync.dma_start(
                out=masks[:].rearrange("p (m a) -> p m a", m=4),
                in_=masks_in.rearrange("m p a -> p m a"),
            )
            cos_t = cpool.tile([128, ST * 64], F32)
            nc.sync.dma_start(
                out=cos_t[:].rearrange("p (t f) -> p t f", t=ST),
                in_=cos_in.rearrange("(t p) f -> p t f", p=128),
            )
            sin_t = cpool.tile([128, ST * 64], F32)
            nc.sync.dma_start(
                out=sin_t[:].rearrange("p (t f) -> p t f", t=ST),
                in_=sin_in.rearrange("(t p) f -> p t f", p=128),
            )
            qw_row = cpool.tile([1, D], F32)
            nc.sync.dma_start(out=qw_row[:], in_=qw_in)
            kw_row = cpool.tile([1, D], F32)
            nc.sync.dma_start(out=kw_row[:], in_=kw_in)
            qW = cpool.tile([128, D], F32)
            nc.gpsimd.partition_broadcast(qW[:], qw_row[:])
            kW = cpool.tile([128, D], F32)
            nc.gpsimd.partition_broadcast(kW[:], kw_row[:])
            eps_t = cpool.tile([128, 1], F32)
            nc.vector.memset(eps_t[:], EPS)
            if reps == -1:
                reps_t = cpool.tile([1, 1], mybir.dt.uint32)
                nc.sync.dma_start(out=reps_t[:], in_=reps_in)
                regs = []
                for e in mybir.ALL_ENGINES:
                    reg = nc.alloc_register(e, f"reps_{e.name}")
                    nc.engines[e].load(reg, reps_t[0:1, 0:1])
                    regs.append(reg)
                reps = bass.RegisterHandles(regs)

            with tc.tile_pool(name="oTp", bufs=1) as oT_pool:
                oT_sb = oT_pool.tile([128, NH * S], BF16)  # [d, head-major s]

                # ============ merged stage 1 + 2 ============
                with (
                    tc.tile_pool(name="qkv", bufs=1) as qkv_pool,
                    tc.tile_pool(name="wqkvp", bufs=1) as wpool,
                    tc.tile_pool(name="xTp", bufs=kn["xa_bufs"]) as xpool,
                    tc.tile_pool(name="s1sb", bufs=2) as s1sb,
                    tc.tile_pool(name="s1stat", bufs=6) as s1stat,
                    tc.tile_pool(name="s2sb", bufs=kn["s2sb_bufs"]) as s2sb,
                    tc.tile_pool(name="s2small", bufs=2) as s2small,
                    tc.tile_pool(name="ps_qkv", bufs=1, space="PSUM") as ps_qkv,
                    tc.tile_pool(name="ps_t", bufs=kn["t_bufs"],
                                 space="PSUM") as ps_t,
                    tc.tile_pool(name="ps_sc", bufs=kn["sc_bufs"],
                                 space="PSUM") as ps_sc,
                    tc.tile_pool(name="ps_o", bufs=kn["o_bufs"], space="PSUM") as ps_o,
                    tc.tile_pool(name="ps_l", bufs=1, space="PSUM") as ps_l,
                    tc.tile_pool(name="ps_b", bufs=1, space="PSUM") as ps_b,
                ):
                    qT_sb = qkv_pool.tile([128, NH * S], F32R)
                    kT_sb = qkv_pool.tile([128, S], F32R)
                    v_sb = qkv_pool.tile([128, S], F32R)

                    wqkv_sb = wpool.tile([128, NK * 768], F32R)

                    def load_wqkv_chunk(ci, ckn):
                        kpc = NK // ckn
                        nc.sync.dma_start(
                            out=wqkv_sb[:, ci * kpc * 768:(ci + 1) * kpc * 768]
                            .rearrange("p (nk n) -> p nk n", nk=kpc),
                            in_=wqkv[ci * kpc * 128:(ci + 1) * kpc * 128, :]
                            .rearrange("(nk p) n -> p nk n", p=128),
                        )

                    def stage1_tile(st):
                        q_ps = ps_qkv.tile([128, 512], F32, tag="q_ps")
                        kv_ps = ps_qkv.tile([128, 256], F32, tag="kv_ps")
                        for kh in range(4):
                            xa = xpool.tile([128, 8 * 128], F32R, tag="xa")
                            nc.sync.dma_start(
                                out=xa[:].rearrange("p (nk m) -> p nk m", nk=8),
                                in_=xT[kh * 1024:(kh + 1) * 1024,
                                       st * 128:(st + 1) * 128]
                                .rearrange("(nk p) m -> p nk m", p=128),
                            )
                            if st == 0:
                                # interleave weight loading with the first
                                # s-tile so TensorE starts immediately
                                load_wqkv_chunk(kh, 4)
                            for kk in range(8):
                                k = kh * 8 + kk
                                lhsT = xa[:, kk * 128:(kk + 1) * 128]
                                nc.tensor.matmul(
                                    q_ps[:], lhsT,
                                    wqkv_sb[:, k * 768:k * 768 + 512],
                                    start=(k == 0), stop=(k == NK - 1),
                                )
                                nc.tensor.matmul(
                                    kv_ps[:], lhsT,
                                    wqkv_sb[:, k * 768 + 512:(k + 1) * 768],
                                    start=(k == 0), stop=(k == NK - 1),
                                )
                        # evacuate psum quickly so the next s-tile can start
                        qkvs = s1sb.tile([128, 512], F32, tag="qkvs")
                        nc.vector.tensor_copy(qkvs[:], q_ps[:])
                        kvs = s1sb.tile([128, 256], F32, tag="kvs")
                        nc.vector.tensor_copy(kvs[:], kv_ps[:])
                        nc.vector.tensor_copy(
                            v_sb[:, st * 128:(st + 1) * 128], kvs[:, 128:256])
                        # rmsnorm + rope + transpose for q blocks + k
                        cs = slice(st * 64, (st + 1) * 64)
                        for blk in range(0 if "epi" in ablate else 5):
                            src = (qkvs[:, blk * 128:(blk + 1) * 128]
                                   if blk < 4 else kvs[:, 0:128])
                            W = qW if blk < 4 else kW
                            sq = s1sb.tile([128, 128], F32, tag="sq")
                            ssq = s1stat.tile([128, 1], F32, tag="ssq")
                            if kn["dve_epi"]:
                                nc.vector.tensor_tensor_reduce(
                                    sq[:], src, src, 1.0, 0.0,
                                    ALU.mult, ALU.add, ssq[:])
                            else:
                                nc.scalar.activation(sq[:], src, AF.Square,
                                                     accum_out=ssq[:])
                            sstd = s1stat.tile([128, 1], F32, tag="sstd")
                            nc.scalar.activation(sstd[:], ssq[:], AF.Sqrt,
                                                 scale=1.0 / D,
                                                 bias=eps_t[:, 0:1])
                            rstd = s1stat.tile([128, 1], F32, tag="rstd")
                            nc.vector.reciprocal(rstd[:], sstd[:])
                            qn = s1sb.tile([128, 128], F32, tag="qn")
                            nc.vector.tensor_tensor(qn[:], src, W[:], ALU.mult)
                            rt = s1sb.tile([128, 128], F32, tag="rt")
                            h1a = s1sb.tile([128, 64], F32, tag="h1a")
                            nc.vector.tensor_tensor(
                                h1a[:], qn[:, 0:64], cos_t[:, cs], ALU.mult)
                            h1b = s1sb.tile([128, 64], F32, tag="h1b")
                            nc.vector.tensor_tensor(
                                h1b[:], qn[:, 64:128], sin_t[:, cs], ALU.mult)
                            nc.vector.tensor_tensor(
                                rt[:, 0:64], h1a[:], h1b[:], ALU.subtract)
                            nc.vector.tensor_tensor(
                                h1a[:], qn[:, 64:128], cos_t[:, cs], ALU.mult)
                            nc.vector.tensor_tensor(
                                h1b[:], qn[:, 0:64], sin_t[:, cs], ALU.mult)
                            nc.vector.tensor_tensor(
                                rt[:, 64:128], h1a[:], h1b[:], ALU.add)
                            rs = s1sb.tile([128, 128], F32, tag="rs")
                            if kn["dve_epi"]:
                                nc.vector.tensor_scalar_mul(rs[:], rt[:],
                                                            rstd[:])
                            else:
                                nc.scalar.activation(rs[:], rt[:], AF.Copy,
                                                     scale=rstd[:])
                            t_ps = ps_t.tile([128, 128], F32, tag="t_ps")
                            nc.tensor.transpose(t_ps[:], rs[:], ident[:])
                            dst = (qT_sb[:, blk * S + st * 128:
                                         blk * S + (st + 1) * 128]
                                   if blk < 4
                                   else kT_sb[:, st * 128:(st + 1) * 128])
                            nc.vector.tensor_copy(dst, t_ps[:])

                    def attn_chunk(c):
                        jlo = max(0, 2 * c - 8)
                        jhi = 2 * c + 1
                        for h in range(NH):
                            o_ps = ps_o.tile([128, 256], F32, tag="o_ps")
                            l_ps = ps_l.tile([1, 256], F32, tag="l_ps")
                            q_sl = qT_sb[:, h * S + c * 256:
                                         h * S + (c + 1) * 256]
                            for j in range(jlo, jhi + 1):
                                sc_ps = ps_sc.tile([128, 256], F32, tag="sc_ps")
                                nc.tensor.matmul(
                                    sc_ps[:], kT_sb[:, j * 128:(j + 1) * 128],
                                    q_sl, start=True, stop=True)
                                pT = s2sb.tile([128, 256], F32R, tag="pT")
                                if "tanh" in ablate:
                                    nc.scalar.activation(
                                        pT[:], sc_ps[:], AF.Exp,
                                        scale=float(SCALE))
                                else:
                                    th = s2sb.tile([128, 256], F32, tag="th")
                                    nc.scalar.activation(
                                        th[:], sc_ps[:], AF.Tanh,
                                        scale=float(SCALE / SOFTCAP))
                                    nc.scalar.activation(
                                        pT[:], th[:], AF.Exp, scale=SOFTCAP)
                                r = j - 2 * c
                                if r in MASK_SLOT:
                                    m = MASK_SLOT[r]
                                    nc.vector.tensor_tensor(
                                        pT[:], pT[:],
                                        masks[:, m * 256:(m + 1) * 256],
                                        ALU.mult)
                                nc.tensor.matmul(
                                    o_ps[:], v_sb[:, j * 128:(j + 1) * 128],
                                    pT[:], start=(j == jlo), stop=(j == jhi))
                                if "sums" not in ablate:
                                    nc.tensor.matmul(
                                        l_ps[:], ones[:, 0:1], pT[:],
                                        start=(j == jlo), stop=(j == jhi))
                            oT_dst = oT_sb[:, h * S + c * 256:
                                           h * S + (c + 1) * 256]
                            if "sums" in ablate:
                                nc.vector.tensor_copy(oT_dst, o_ps[:])
                            elif kn["gp_bcast"]:
                                rec = s2small.tile([1, 256], F32, tag="rec")
                                nc.vector.reciprocal(rec[:], l_ps[:])
                                bc = s2small.tile([128, 256], F32, tag="bc")
                                nc.gpsimd.partition_broadcast(bc[:], rec[:])
                                nc.vector.tensor_tensor(
                                    oT_dst, o_ps[:], bc[:], ALU.mult)
                            else:
                                rec = s2small.tile([1, 256], F32R, tag="recr")
                                with nc.allow_low_precision(reason="tf32-ish"):
                                    nc.vector.reciprocal(rec[:], l_ps[:])
                                b_ps = ps_b.tile([128, 256], F32, tag="b_ps")
                                nc.tensor.matmul(b_ps[:], ones[0:1, :], rec[:],
                                                 start=True, stop=True)
                                bc = s2small.tile([128, 256], F32, tag="bc")
                                nc.scalar.copy(bc[:], b_ps[:])
                                nc.vector.tensor_tensor(
                                    oT_dst, o_ps[:], bc[:], ALU.mult)

                    def merged_body():
                        for st in range(ST):
                            if 1 in stages:
                                stage1_tile(st)
                            if st % 2 == 1 and 2 in stages:
                                c = st // 2
                                attn_chunk(c)
                                if 3 in stages:
                                    # stage a2a input for this finished chunk
                                    nc.sync.dma_start(
                                        out=a2a_in[c].rearrange(
                                            "(h p) s -> p h s", p=128),
                                        in_=oT_sb[:].rearrange(
                                            "p (h s) -> p h s", h=NH)
                                        [:, :, c * SSH:(c + 1) * SSH],
                                    )

                    if reps:
                        with tc.For_i(0, reps, 1):
                            merged_body()
                    else:
                        merged_body()

            # ================== stage 3 ==================
            with (
                tc.tile_pool(name="wop", bufs=kn["wo_bufs"]) as wopool,
                tc.tile_pool(name="oTfp", bufs=1) as oTf_pool,
                tc.tile_pool(name="outstp", bufs=2) as outst_pool,
            ):
                if 3 in stages:
                    if sim_mode:
                        nc.sync.dma_start(out=a2a_out[:], in_=a2a_in[:])
                    else:
                        nc.gpsimd.collective_compute(
                            "AllToAll", ALU.bypass,
                            replica_groups=[list(range(NC))],
                            ins=[a2a_in[:]], outs=[a2a_out[:]],
                        )
                oTf = oTf_pool.tile([128, NK * SSH], BF16)
                if 3 in stages:
                    a2a_flat = a2a_out.rearrange("r d s -> (r d) s")
                    for qi in range(4):
                        kq = NK // 4
                        nc.sync.dma_start(
                            out=oTf[:, qi * kq * SSH:(qi + 1) * kq * SSH]
                            .rearrange("p (kd s) -> p kd s", kd=kq),
                            in_=a2a_flat[qi * kq * 128:(qi + 1) * kq * 128, :]
                            .rearrange("(kd p) s -> p kd s", p=128),
                        )

                with tc.tile_pool(name="ps3", bufs=1, space="PSUM") as ps3:
                    def stage3_body():
                        for nh in range(2):
                            o3_a = ps3.tile([128, 2048], F32, tag="o3_a")
                            o3_b = ps3.tile([128, 2048], F32, tag="o3_b")
                            out_ps = [o3_a, o3_b]
                            for kd in range(NK):
                                wo_t = wopool.tile([128, 2048], BF16, tag="wo")
                                nc.sync.dma_start(
                                    out=wo_t[:],
                                    in_=wo[kd * 128:(kd + 1) * 128,
                                           nh * 2048:(nh + 1) * 2048],
                                )
                                for sti in range(2):
                                    lhsT = oTf[:, kd * SSH + sti * 128:
                                               kd * SSH + (sti + 1) * 128]
                                    for ncn in range(4):
                                        nc.tensor.matmul(
                                            out_ps[sti][:, ncn * 512:
                                                        (ncn + 1) * 512],
                                            lhsT,
                                            wo_t[:, ncn * 512:(ncn + 1) * 512],
                                            start=(kd == 0),
                                            stop=(kd == NK - 1))
                            for sti in range(2):
                                ost = outst_pool.tile([128, 2048], F32,
                                                      tag="ost")
                                nc.vector.tensor_copy(ost[:], out_ps[sti][:])
                                nc.sync.dma_start(
                                    out=out_shard[sti * 128:(sti + 1) * 128,
                                                  nh * 2048:(nh + 1) * 2048],
                                    in_=ost[:])
                                if tiny_out is not None:
                                    nc.sync.dma_start(
                                        out=tiny_out[:, (nh * 2 + sti) * 16:
                                                     (nh * 2 + sti + 1) * 16],
                                        in_=ost[0:16, 0:16])

                    if 3 in stages:
                        if reps:
                            with tc.For_i(0, reps, 1):
                                stage3_body()
                        else:
                            stage3_body()

    nc.compile()
    return nc


def _prepare_in_maps(x, wq, wk, wv, wo, q_norm_w, k_norm_w):
    import ml_dtypes
    xT = _round_f32r(np.ascontiguousarray(x.reshape(S, H).T))
    wo_r = np.ascontiguousarray(wo).astype(ml_dtypes.bfloat16)
    cos_np, sin_np = _rope_tables()
    masks_np = _mask_tiles()
    ones_np = np.ones((128, 128), np.float32)
    qw = np.ascontiguousarray(q_norm_w.reshape(1, D)).astype(np.float32)
    kw = np.ascontiguousarray(k_norm_w.reshape(1, D)).astype(np.float32)
    in_maps = []
    for c in range(NC):
        wqkv_c = np.concatenate(
            [wq[:, c * 512:(c + 1) * 512],
             wk[:, c * 128:(c + 1) * 128],
             wv[:, c * 128:(c + 1) * 128]], axis=1)
        in_maps.append({
            "xT": xT,
            "wqkv": _round_f32r(np.ascontiguousarray(wqkv_c)),
            "wo": wo_r,
            "cos_in": cos_np, "sin_in": sin_np,
            "masks_in": masks_np,
            "qw_in": qw, "kw_in": kw,
            "ones_in": ones_np,
        })
    return in_maps


_PROGRAM_CACHE = {}


def kernel(x, wq, wk, wv, wo, q_norm_w, k_norm_w):
    x = np.asarray(x, dtype=np.float32)
    in_maps = _prepare_in_maps(
        x, np.asarray(wq, np.float32), np.asarray(wk, np.float32),
        np.asarray(wv, np.float32), np.asarray(wo, np.float32),
        np.asarray(q_norm_w, np.float32), np.asarray(k_norm_w, np.float32))
    if "p" not in _PROGRAM_CACHE:
        _PROGRAM_CACHE["p"] = build_program(reps=0)
    nc = _PROGRAM_CACHE["p"]
    res = run_bass_kernel_spmd(nc, in_maps, list(range(NC)))
    out = np.concatenate([res.results[c]["out_shard"] for c in range(NC)], axis=0)
    return out.reshape(B, S, H)


# revision 45
# speedup vs baseline: 1.0485x; 1.0485x over previous
"""Trainium2 Bass kernel for nn_Attention_49005576847767.

GQA attention block (QKV proj + Q/K RMSNorm + NeoX RoPE + sliding-window
causal attention with tanh softcap + output proj), tensor-parallel over
heads across 8 NeuronCores.

Sharding: core c owns KV head c and query heads 4c..4c+3.
  Merged stage 1+2: per 128-row s-tile, QKV projection (fp32r matmuls),
    RMSNorm + RoPE epilogue, PE transposes -> qT/kT/v; after every odd
    s-tile, flash-style attention for the finished 256-row q-chunk with
    *transposed* scores [s_k, s_q] (softcap bounds scores at +-50 so no
    max-subtraction is needed; row sums via a ones-column matmul).
    Interleaving keeps TensorE busy while ScalarE does tanh/exp.
  Stage 3: AllToAll reshards o from head-split to sequence-split, then
    each core computes its 256 output rows against the full wo (bf16).
Host assembles the 8 row-shards.
"""

import numpy as np

import concourse.bass as bass
import concourse.mybir as mybir
import concourse.tile as tile
from concourse import bacc
from concourse.bass_utils import run_bass_kernel_spmd
from concourse.masks import make_identity

F32 = mybir.dt.float32
F32R = mybir.dt.float32r
BF16 = mybir.dt.bfloat16
AF = mybir.ActivationFunctionType
ALU = mybir.AluOpType

# problem shapes (hardcoded per contract)
B, S, H = 1, 2048, 4096
HQ, HKV, D = 32, 8, 128
NC = 8                 # cores
NH = HQ // NC          # 4 query heads per core
WINDOW = 1024
SOFTCAP = 50.0
EPS = 1e-6
THETA = 10000.0
SCALE = 1.0 / float(np.sqrt(np.float32(D)))

ST = S // 128          # 16 s-tiles
NK = H // 128          # 32 contraction tiles for projections
CH = S // 256          # 8 q-chunks of 256 rows
SSH = S // NC          # 256 output rows per core

MASK_SLOT = {-8: 0, -7: 1, 0: 2, 1: 3}


def _round_f32r(x: np.ndarray) -> np.ndarray:
    """Round fp32 to the PE's fp32r format (RNE, 12 mantissa bits dropped)."""
    u = np.ascontiguousarray(x.astype(np.float32)).view(np.uint32)
    u = (u + 0x7FF + ((u >> 12) & 1)) & np.uint32(0xFFFFF000)
    return u.view(np.float32)


def _rope_tables():
    half = D // 2
    inv_freq = 1.0 / (THETA ** (np.arange(half, dtype=np.float64) / half))
    ang = np.arange(S, dtype=np.float64)[:, None] * inv_freq[None, :]
    return (np.cos(ang).astype(np.float32), np.sin(ang).astype(np.float32))


def _mask_tiles() -> np.ndarray:
    """[4, 128, 256] multiplicative masks for relative k-tile offsets
    r in {-8, -7, 0, +1} of a 256-wide q-chunk. Entry [b, a] valid iff
    0 <= a - b - 128 r <= WINDOW."""
    b = np.arange(128)[:, None]
    a = np.arange(256)[None, :]
    out = np.zeros((4, 128, 256), np.float32)
    for idx, r in enumerate((-8, -7, 0, 1)):
        d = a - b - 128 * r
        out[idx] = ((d >= 0) & (d <= WINDOW)).astype(np.float32)
    return out


def build_program(reps: int = 0, sim_mode: bool = False, stages=(1, 2, 3),
                  timing_mode: bool = False, ablate=frozenset(), knobs=None):
    """Build the SPMD program. reps=0 -> straight-line (graded path);
    reps=N>0 -> static hardware loops; reps=-1 -> loop count read from a
    uint32 input at runtime (timing). sim_mode -> single-core, collective
    replaced by a local DMA, for cost-model runs."""
    stages = set(stages)
    kn = {"xa_bufs": 2, "sc_bufs": 2, "s2sb_bufs": 3, "wo_bufs": 12,
          "wqkv_chunks": 8, "gp_bcast": True, "dve_epi": False, "t_bufs": 1, "o_bufs": 2}
    kn.update(knobs or {})
    nc = bacc.Bacc("TRN2", target_bir_lowering=False, debug=False,
                   num_devices=1 if sim_mode else NC)

    if timing_mode:
        # garbage-valued internal tensors: no host->device transfer, so
        # per-call wall is RTT + R * kernel-time (values don't affect timing)
        xT = nc.dram_tensor("xT", [H, S], F32R).ap()
        wqkv = nc.dram_tensor("wqkv", [H, 768], F32R).ap()
        wo = nc.dram_tensor("wo", [H, H], BF16).ap()
    else:
        xT = nc.dram_tensor("xT", [H, S], F32R, kind="ExternalInput").ap()
        wqkv = nc.dram_tensor("wqkv", [H, 768], F32R, kind="ExternalInput").ap()
        wo = nc.dram_tensor("wo", [H, H], BF16, kind="ExternalInput").ap()
    cos_in = nc.dram_tensor("cos_in", [S, 64], F32, kind="ExternalInput").ap()
    sin_in = nc.dram_tensor("sin_in", [S, 64], F32, kind="ExternalInput").ap()
    masks_in = nc.dram_tensor("masks_in", [4, 128, 256], F32R,
                              kind="ExternalInput").ap()
    qw_in = nc.dram_tensor("qw_in", [1, D], F32, kind="ExternalInput").ap()
    kw_in = nc.dram_tensor("kw_in", [1, D], F32, kind="ExternalInput").ap()
    ones_in = nc.dram_tensor("ones_in", [128, 128], F32R,
                             kind="ExternalInput").ap()
    if reps == -1:
        reps_in = nc.dram_tensor("reps_in", [1, 1], mybir.dt.uint32,
                                 kind="ExternalInput").ap()
    if timing_mode:
        out_shard = nc.dram_tensor("out_shard", [SSH, H], F32).ap()
        tiny_out = nc.dram_tensor("tiny_out", [16, 64], F32,
                                  kind="ExternalOutput").ap()
    else:
        out_shard = nc.dram_tensor("out_shard", [SSH, H], F32,
                                   kind="ExternalOutput").ap()
        tiny_out = None

    a2a_in = nc.dram_tensor("a2a_in", [NC, NH * D, SSH], BF16)
    a2a_out = nc.dram_tensor("a2a_out", [NC, NH * D, SSH], BF16)

    with tile.TileContext(nc) as tc:
        with tc.tile_pool(name="const", bufs=1) as cpool:
            # ---- constants (~15KB/partition) ----
            ident = cpool.tile([128, 128], F32)
            make_identity(nc, ident[:])
            ones = cpool.tile([128, 128], F32R)
            nc.sync.dma_start(out=ones[:], in_=ones_in)
            masks = cpool.tile([128, 4 * 256], F32R)
            nc.sync.dma_start(
                out=masks[:].rearrange("p (m a) -> p m a", m=4),
                in_=masks_in.rearrange("m p a -> p m a"),
            )
            cos_t = cpool.tile([128, ST * 64], F32)
            nc.sync.dma_start(
                out=cos_t[:].rearrange("p (t f) -> p t f", t=ST),
                in_=cos_in.rearrange("(t p) f -> p t f", p=128),
            )
            sin_t = cpool.tile([128, ST * 64], F32)
            nc.sync.dma_start(
                out=sin_t[:].rearrange("p (t f) -> p t f", t=ST),
                in_=sin_in.rearrange("(t p) f -> p t f", p=128),
            )
            qw_row = cpool.tile([1, D], F32)
            nc.sync.dma_start(out=qw_row[:], in_=qw_in)
            kw_row = cpool.tile([1, D], F32)
            nc.sync.dma_start(out=kw_row[:], in_=kw_in)
            qW = cpool.tile([128, D], F32)
            nc.gpsimd.partition_broadcast(qW[:], qw_row[:])
            kW = cpool.tile([128, D], F32)
            nc.gpsimd.partition_broadcast(kW[:], kw_row[:])
            eps_t = cpool.tile([128, 1], F32)
            nc.vector.memset(eps_t[:], EPS)
            if reps == -1:
                reps_t = cpool.tile([1, 1], mybir.dt.uint32)
                nc.sync.dma_start(out=reps_t[:], in_=reps_in)
                regs = []
                for e in mybir.ALL_ENGINES:
                    reg = nc.alloc_register(e, f"reps_{e.name}")
                    nc.engines[e].load(reg, reps_t[0:1, 0:1])
                    regs.append(reg)
                reps = bass.RegisterHandles(regs)

            with tc.tile_pool(name="oTp", bufs=1) as oT_pool:
                oT_sb = oT_pool.tile([128, NH * S], BF16)  # [d, head-major s]

                # ============ merged stage 1 + 2 ============
                with (
                    tc.tile_pool(name="qkv", bufs=1) as qkv_pool,
                    tc.tile_pool(name="wqkvp", bufs=1) as wpool,
                    tc.tile_pool(name="xTp", bufs=kn["xa_bufs"]) as xpool,
                    tc.tile_pool(name="s1sb", bufs=2) as s1sb,
                    tc.tile_pool(name="s1stat", bufs=6) as s1stat,
                    tc.tile_pool(name="s2sb", bufs=kn["s2sb_bufs"]) as s2sb,
                    tc.tile_pool(name="s2small", bufs=2) as s2small,
                    tc.tile_pool(name="ps_qkv", bufs=1, space="PSUM") as ps_qkv,
                    tc.tile_pool(name="ps_t", bufs=kn["t_bufs"],
                                 space="PSUM") as ps_t,
                    tc.tile_pool(name="ps_sc", bufs=kn["sc_bufs"],
                                 space="PSUM") as ps_sc,
                    tc.tile_pool(name="ps_o", bufs=kn["o_bufs"], space="PSUM") as ps_o,
                    tc.tile_pool(name="ps_l", bufs=1, space="PSUM") as ps_l,
                    tc.tile_pool(name="ps_b", bufs=1, space="PSUM") as ps_b,
                ):
                    qT_sb = qkv_pool.tile([128, NH * S], F32R)
                    kT_sb = qkv_pool.tile([128, S], F32R)
                    v_sb = qkv_pool.tile([128, S], F32R)

                    wqkv_sb = wpool.tile([128, NK * 768], F32R)

                    def load_wqkv_chunk(ci, ckn):
                        kpc = NK // ckn
                        nc.sync.dma_start(
                            out=wqkv_sb[:, ci * kpc * 768:(ci + 1) * kpc * 768]
                            .rearrange("p (nk n) -> p nk n", nk=kpc),
                            in_=wqkv[ci * kpc * 128:(ci + 1) * kpc * 128, :]
                            .rearrange("(nk p) n -> p nk n", p=128),
                        )

                    def stage1_tile(st):
                        q_ps = ps_qkv.tile([128, 512], F32, tag="q_ps")
                        kv_ps = ps_qkv.tile([128, 256], F32, tag="kv_ps")
                        for kh in range(4):
                            xa = xpool.tile([128, 8 * 128], F32R, tag="xa")
                            nc.sync.dma_start(
                                out=xa[:].rearrange("p (nk m) -> p nk m", nk=8),
                                in_=xT[kh * 1024:(kh + 1) * 1024,
                                       st * 128:(st + 1) * 128]
                                .rearrange("(nk p) m -> p nk m", p=128),
                            )
                            if st == 0:
                                # interleave weight loading with the first
                                # s-tile so TensorE starts immediately
                                load_wqkv_chunk(kh, 4)
                            for kk in range(8):
                                k = kh * 8 + kk
                                lhsT = xa[:, kk * 128:(kk + 1) * 128]
                                nc.tensor.matmul(
                                    q_ps[:], lhsT,
                                    wqkv_sb[:, k * 768:k * 768 + 512],
                                    start=(k == 0), stop=(k == NK - 1),
                                )
                                nc.tensor.matmul(
                                    kv_ps[:], lhsT,
                                    wqkv_sb[:, k * 768 + 512:(k + 1) * 768],
                                    start=(k == 0), stop=(k == NK - 1),
                                )
                        # evacuate psum quickly so the next s-tile can start
                        qkvs = s1sb.tile([128, 512], F32, tag="qkvs")
                        nc.vector.tensor_copy(qkvs[:], q_ps[:])
                        kvs = s1sb.tile([128, 256], F32, tag="kvs")
                        nc.vector.tensor_copy(kvs[:], kv_ps[:])
                        nc.vector.tensor_copy(
                            v_sb[:, st * 128:(st + 1) * 128], kvs[:, 128:256])
                        # rmsnorm + rope + transpose for q blocks + k
                        cs = slice(st * 64, (st + 1) * 64)
                        for blk in range(0 if "epi" in ablate else 5):
                            src = (qkvs[:, blk * 128:(blk + 1) * 128]
                                   if blk < 4 else kvs[:, 0:128])
                            W = qW if blk < 4 else kW
                            sq = s1sb.tile([128, 128], F32, tag="sq")
                            ssq = s1stat.tile([128, 1], F32, tag="ssq")
                            if kn["dve_epi"]:
                                nc.vector.tensor_tensor_reduce(
                                    sq[:], src, src, 1.0, 0.0,
                                    ALU.mult, ALU.add, ssq[:])
                            else:
                                nc.scalar.activation(sq[:], src, AF.Square,
                                                     accum_out=ssq[:])
                            sstd = s1stat.tile([128, 1], F32, tag="sstd")
                            nc.scalar.activation(sstd[:], ssq[:], AF.Sqrt,
                                                 scale=1.0 / D,
                                                 bias=eps_t[:, 0:1])
                            rstd = s1stat.tile([128, 1], F32, tag="rstd")
                            nc.vector.reciprocal(rstd[:], sstd[:])
                            qn = s1sb.tile([128, 128], F32, tag="qn")
                            nc.vector.tensor_tensor(qn[:], src, W[:], ALU.mult)
                            rt = s1sb.tile([128, 128], F32, tag="rt")
                            h1a = s1sb.tile([128, 64], F32, tag="h1a")
                            nc.vector.tensor_tensor(
                                h1a[:], qn[:, 0:64], cos_t[:, cs], ALU.mult)
                            h1b = s1sb.tile([128, 64], F32, tag="h1b")
                            nc.vector.tensor_tensor(
                                h1b[:], qn[:, 64:128], sin_t[:, cs], ALU.mult)
                            nc.vector.tensor_tensor(
                                rt[:, 0:64], h1a[:], h1b[:], ALU.subtract)
                            nc.vector.tensor_tensor(
                                h1a[:], qn[:, 64:128], cos_t[:, cs], ALU.mult)
                            nc.vector.tensor_tensor(
                                h1b[:], qn[:, 0:64], sin_t[:, cs], ALU.mult)
                            nc.vector.tensor_tensor(
                                rt[:, 64:128], h1a[:], h1b[:], ALU.add)
                            rs = s1sb.tile([128, 128], F32, tag="rs")
                            if kn["dve_epi"]:
                                nc.vector.tensor_scalar_mul(rs[:], rt[:],
                                                            rstd[:])
                            else:
                                nc.scalar.activation(rs[:], rt[:], AF.Copy,
                                                     scale=rstd[:])
                            t_ps = ps_t.tile([128, 128], F32, tag="t_ps")
                            nc.tensor.transpose(t_ps[:], rs[:], ident[:])
                            dst = (qT_sb[:, blk * S + st * 128:
                                         blk * S + (st + 1) * 128]
                                   if blk < 4
                                   else kT_sb[:, st * 128:(st + 1) * 128])
                            nc.vector.tensor_copy(dst, t_ps[:])

                    def attn_chunk(c):
                        jlo = max(0, 2 * c - 8)
                        jhi = 2 * c + 1
                        for h in range(NH):
                            o_ps = ps_o.tile([128, 256], F32, tag="o_ps")
                            l_ps = ps_l.tile([1, 256], F32, tag="l_ps")
                            q_sl = qT_sb[:, h * S + c * 256:
                                         h * S + (c + 1) * 256]
                            for j in range(jlo, jhi + 1):
                                sc_ps = ps_sc.tile([128, 256], F32, tag="sc_ps")
                                nc.tensor.matmul(
                                    sc_ps[:], kT_sb[:, j * 128:(j + 1) * 128],
                                    q_sl, start=True, stop=True)
                                pT = s2sb.tile([128, 256], F32R, tag="pT")
                                if "tanh" in ablate:
                                    nc.scalar.activation(
                                        pT[:], sc_ps[:], AF.Exp,
                                        scale=float(SCALE))
                                else:
                                    th = s2sb.tile([128, 256], F32, tag="th")
                                    nc.scalar.activation(
                                        th[:], sc_ps[:], AF.Tanh,
                                        scale=float(SCALE / SOFTCAP))
                                    nc.scalar.activation(
                                        pT[:], th[:], AF.Exp, scale=SOFTCAP)
                                r = j - 2 * c
                                if r in MASK_SLOT:
                                    m = MASK_SLOT[r]
                                    nc.vector.tensor_tensor(
                                        pT[:], pT[:],
                                        masks[:, m * 256:(m + 1) * 256],
                                        ALU.mult)
                                nc.tensor.matmul(
                                    o_ps[:], v_sb[:, j * 128:(j + 1) * 128],
                                    pT[:], start=(j == jlo), stop=(j == jhi))
                                if "sums" not in ablate:
                                    nc.tensor.matmul(
                                        l_ps[:], ones[:, 0:1], pT[:],
                                        start=(j == jlo), stop=(j == jhi))
                            oT_dst = oT_sb[:, h * S + c * 256:
                                           h * S + (c + 1) * 256]
                            if "sums" in ablate:
                                nc.vector.tensor_copy(oT_dst, o_ps[:])
                            elif kn["gp_bcast"]:
                                rec = s2small.tile([1, 256], F32, tag="rec")
                                nc.vector.reciprocal(rec[:], l_ps[:])
                                bc = s2small.tile([128, 256], F32, tag="bc")
                                nc.gpsimd.partition_broadcast(bc[:], rec[:])
                                nc.vector.tensor_tensor(
                                    oT_dst, o_ps[:], bc[:], ALU.mult)
                            else:
                                rec = s2small.tile([1, 256], F32R, tag="recr")
                                with nc.allow_low_precision(reason="tf32-ish"):
                                    nc.vector.reciprocal(rec[:], l_ps[:])
                                b_ps = ps_b.tile([128, 256], F32, tag="b_ps")
                                nc.tensor.matmul(b_ps[:], ones[0:1, :], rec[:],
                                                 start=True, stop=True)
                                bc = s2small.tile([128, 256], F32, tag="bc")
                                nc.scalar.copy(bc[:], b_ps[:])
                                nc.vector.tensor_tensor(
                                    oT_dst, o_ps[:], bc[:], ALU.mult)

                    def merged_body():
                        for st in range(ST):
                            if 1 in stages:
                                stage1_tile(st)
                            if st % 2 == 1 and 2 in stages:
                                c = st // 2
                                attn_chunk(c)
                                if 3 in stages:
                                    # stage a2a input for this finished chunk
                                    nc.sync.dma_start(
                                        out=a2a_in[c].rearrange(
                                            "(h p) s -> p h s", p=128),
                                        in_=oT_sb[:].rearrange(
                                            "p (h s) -> p h s", h=NH)
                                        [:, :, c * SSH:(c + 1) * SSH],
                                    )

                    if reps:
                        with tc.For_i(0, reps, 1):
                            merged_body()
                    else:
                        merged_body()

            # ================== stage 3 ==================
            with (
                tc.tile_pool(name="wop", bufs=kn["wo_bufs"]) as wopool,
                tc.tile_pool(name="oTfp", bufs=1) as oTf_pool,
                tc.tile_pool(name="outstp", bufs=2) as outst_pool,
            ):
                if 3 in stages:
                    if sim_mode:
                        nc.sync.dma_start(out=a2a_out[:], in_=a2a_in[:])
                    else:
                        nc.gpsimd.collective_compute(
                            "AllToAll", ALU.bypass,
                            replica_groups=[list(range(NC))],
                            ins=[a2a_in[:]], outs=[a2a_out[:]],
                        )
                oTf = oTf_pool.tile([128, NK * SSH], BF16)
                if 3 in stages:
                    a2a_flat = a2a_out.rearrange("r d s -> (r d) s")
                    for qi in range(4):
                        kq = NK // 4
                        nc.sync.dma_start(
                            out=oTf[:, qi * kq * SSH:(qi + 1) * kq * SSH]
                            .rearrange("p (kd s) -> p kd s", kd=kq),
                            in_=a2a_flat[qi * kq * 128:(qi + 1) * kq * 128, :]
                            .rearrange("(kd p) s -> p kd s", p=128),
                        )

                with tc.tile_pool(name="ps3", bufs=1, space="PSUM") as ps3:
                    def stage3_body():
                        for nh in range(2):
                            o3_a = ps3.tile([128, 2048], F32, tag="o3_a")
                            o3_b = ps3.tile([128, 2048], F32, tag="o3_b")
                            out_ps = [o3_a, o3_b]
                            for kd in range(NK):
                                wo_t = wopool.tile([128, 2048], BF16, tag="wo")
                                nc.sync.dma_start(
                                    out=wo_t[:],
                                    in_=wo[kd * 128:(kd + 1) * 128,
                                           nh * 2048:(nh + 1) * 2048],
                                )
                                for sti in range(2):
                                    lhsT = oTf[:, kd * SSH + sti * 128:
                                               kd * SSH + (sti + 1) * 128]
                                    for ncn in range(4):
                                        nc.tensor.matmul(
                                            out_ps[sti][:, ncn * 512:
                                                        (ncn + 1) * 512],
                                            lhsT,
                                            wo_t[:, ncn * 512:(ncn + 1) * 512],
                                            start=(kd == 0),
                                            stop=(kd == NK - 1))
                            for sti in range(2):
                                for ei in range(2):
                                    ost = outst_pool.tile([128, 1024], F32,
                                                          tag="ost")
                                    nc.vector.tensor_copy(
                                        ost[:],
                                        out_ps[sti][:, ei * 1024:
                                                     (ei + 1) * 1024])
                                    nc.sync.dma_start(
                                        out=out_shard[
                                            sti * 128:(sti + 1) * 128,
                                            nh * 2048 + ei * 1024:
                                            nh * 2048 + (ei + 1) * 1024],
                                        in_=ost[:])
                                    if tiny_out is not None and ei == 0:
                                        nc.sync.dma_start(
                                            out=tiny_out[
                                                :, (nh * 2 + sti) * 16:
                                                (nh * 2 + sti + 1) * 16],
                                            in_=ost[0:16, 0:16])

                    if 3 in stages:
                        if reps:
                            with tc.For_i(0, reps, 1):
                                stage3_body()
                        else:
                            stage3_body()

    nc.compile()
    return nc


def _prepare_in_maps(x, wq, wk, wv, wo, q_norm_w, k_norm_w):
    import ml_dtypes
    xT = _round_f32r(np.ascontiguousarray(x.reshape(S, H).T))
    wo_r = np.ascontiguousarray(wo).astype(ml_dtypes.bfloat16)
    cos_np, sin_np = _rope_tables()
    masks_np = _mask_tiles()
    ones_np = np.ones((128, 128), np.float32)
    qw = np.ascontiguousarray(q_norm_w.reshape(1, D)).astype(np.float32)
    kw = np.ascontiguousarray(k_norm_w.reshape(1, D)).astype(np.float32)
    in_maps = []
    for c in range(NC):
        wqkv_c = np.concatenate(
            [wq[:, c * 512:(c + 1) * 512],
             wk[:, c * 128:(c + 1) * 128],
             wv[:, c * 128:(c + 1) * 128]], axis=1)
        in_maps.append({
            "xT": xT,
            "wqkv": _round_f32r(np.ascontiguousarray(wqkv_c)),
            "wo": wo_r,
            "cos_in": cos_np, "sin_in": sin_np,
            "masks_in": masks_np,
            "qw_in": qw, "kw_in": kw,
            "ones_in": ones_np,
        })
    return in_maps


_PROGRAM_CACHE = {}


def kernel(x, wq, wk, wv, wo, q_norm_w, k_norm_w):
    x = np.asarray(x, dtype=np.float32)
    in_maps = _prepare_in_maps(
        x, np.asarray(wq, np.float32), np.asarray(wk, np.float32),
        np.asarray(wv, np.float32), np.asarray(wo, np.float32),
        np.asarray(q_norm_w, np.float32), np.asarray(k_norm_w, np.float32))
    if "p" not in _PROGRAM_CACHE:
        _PROGRAM_CACHE["p"] = build_program(reps=0)
    nc = _PROGRAM_CACHE["p"]
    res = run_bass_kernel_spmd(nc, in_maps, list(range(NC)))
    out = np.concatenate([res.results[c]["out_shard"] for c in range(NC)], axis=0)
    return out.reshape(B, S, H)
